# revision 1
# baseline (speedup 1.0000x reference)
"""Trainium2 Bass kernel for AuditableHybridGNN (gnn_message_passing).

Computation (reference):
  h_ent = LN((1-a)*h_local + a*h_global) * gamma_e + beta_e        [100000,256]
  rel   = sigmoid(sum(h_ent * q, -1))                              [100000]
  ctx   = segment_sum(h_ent[ent_idx] * rel[ent_idx], psg_idx)      [20000,256]
  h_psg = LN(h_passage + ctx) * gamma_p + beta_p                   [20000,256]
  out   = relu([h_psg, q] @ W1 + b1) @ W2 + b2                     [20000]

Distribution over 8 NeuronCores (SPMD, one program):
  - entities sharded 12500/core: each core computes g = h_ent*rel for its
    shard and writes the table (bf16) to its DRAM;
  - edges sharded by entity owner; per core, edges are bucketed on host by
    125-passage block (fixed T_B tiles of 128 per block); device gathers edge
    rows with dma_gather (1024 idx/instr, rotating over 4 SWDGE queues),
    builds a one-hot [edge x psg_local] selection matrix on DVE and
    matmul-accumulates per-block partials in PSUM -> [20000,256] partial;
  - blocks are processed in (chunk, rank, j) order so 4 chunked
    ReduceScatter(add) collectives fire early and overlap the edge phase;
  - each core ends with its 2500 passages; LN + scoring MLP -> [2500]
    scores; host concatenates.
"""
import sys

sys.path.insert(0, "/opt/trn_rl_repo")

import numpy as np

import concourse.bass as bass
from concourse import bacc, mybir, tile
from concourse.masks import make_identity

F32 = mybir.dt.float32
BF16 = mybir.dt.bfloat16
I16 = mybir.dt.int16

NCORES = 8
N_ENT = 100000
N_PSG = 20000
N_EDGE = 500000
D = 256
ALPHA = 0.1
EPS = 1e-5

ENT_SH = N_ENT // NCORES  # 12500
PSG_SH = N_PSG // NCORES  # 2500
ENT_TILES = (ENT_SH + 127) // 128  # 98
ENT_PAD = ENT_TILES * 128  # 12544
PSG_TILES = (PSG_SH + 127) // 128  # 20
PSG_PAD = PSG_TILES * 128  # 2560
BLK = 125  # passages per block (aligns blocks with rank shards: 2500 = 20*125)
NBLK = N_PSG // BLK  # 160
NCHUNK = 4  # chunked reduce-scatters; each chunk = 5 blocks/rank = 625 psg/rank
BLK_PER_CHUNK = NBLK // NCHUNK  # 40
GBLK = 2  # blocks per dma_gather (with t_b=4 -> 1024 idx, the SWDGE ring max)
NQ = 4  # SWDGE queues for gathers
ECHUNK = 4  # entity tiles per DMA batch


def build_nc(reps, t_b, use_gamma_e, use_beta_e, use_gamma_p, use_beta_p, use_b2,
             phases=(1, 1, 1, 1)):
    nc = bacc.Bacc(
        "TRN2",
        target_bir_lowering=False,
        debug=False,
        num_devices=NCORES,
        num_swdge_queues=NQ,
    )
    n_slots = NBLK * t_b * 128

    hl_ext = nc.dram_tensor("hl", [ENT_PAD, D], F32, kind="ExternalInput")
    hg_ext = nc.dram_tensor("hg", [ENT_PAD, D], F32, kind="ExternalInput")
    hp_ext = nc.dram_tensor("hp", [PSG_PAD, D], F32, kind="ExternalInput")
    q_ext = nc.dram_tensor("q", [1, D], F32, kind="ExternalInput")
    idx_ext = nc.dram_tensor("idx16", [128, n_slots // 16], I16, kind="ExternalInput")
    ploc_ext = nc.dram_tensor("ploc", [128, n_slots // 128], F32, kind="ExternalInput")
    iota_ext = nc.dram_tensor("iota", [128, 128], F32, kind="ExternalInput")
    w1_ext = nc.dram_tensor("w1", [D, D], F32, kind="ExternalInput")
    qb1_ext = nc.dram_tensor("qb1", [128, 2], F32, kind="ExternalInput")
    w2_ext = nc.dram_tensor("w2", [128, 2], F32, kind="ExternalInput")
    b2_ext = nc.dram_tensor("b2", [1, 1], F32, kind="ExternalInput")
    ge_ext = nc.dram_tensor("gamma_e", [1, D], F32, kind="ExternalInput")
    be_ext = nc.dram_tensor("beta_e", [1, D], F32, kind="ExternalInput")
    gp_ext = nc.dram_tensor("gamma_p", [1, D], F32, kind="ExternalInput")
    bp_ext = nc.dram_tensor("beta_p", [1, D], F32, kind="ExternalInput")
    out_ext = nc.dram_tensor("out", [1, PSG_PAD], F32, kind="ExternalOutput")

    hl_r = hl_ext[:].rearrange("(c p) d -> p c d", p=128)
    hg_r = hg_ext[:].rearrange("(c p) d -> p c d", p=128)

    with tile.TileContext(nc) as tc:
        with (
            tc.tile_pool(name="consts", bufs=1) as consts,
            tc.tile_pool(name="ent", bufs=3) as ent,
            tc.tile_pool(name="ent_sc", bufs=4) as ent_sc,
            tc.tile_pool(name="gatp", bufs=6) as gatp,
            tc.tile_pool(name="ohp", bufs=4) as ohp,
            tc.tile_pool(name="evp", bufs=4) as evp,
            tc.tile_pool(name="mlp", bufs=3) as mlp,
            tc.tile_pool(name="eps_ps", bufs=3, space="PSUM") as eps_ps,
            tc.tile_pool(name="tp_ps", bufs=2, space="PSUM") as tp_ps,
            tc.tile_pool(name="hid_ps", bufs=2, space="PSUM") as hid_ps,
            tc.tile_pool(name="sc_ps", bufs=1, space="PSUM") as sc_ps,
            tc.tile_pool(name="dram", bufs=1, space="DRAM") as dram,
        ):
            # ---- constants loaded once ----
            idx_t = consts.tile([128, n_slots // 16], I16)
            nc.sync.dma_start(out=idx_t[:], in_=idx_ext[:])
            ploc_t = consts.tile([128, n_slots // 128], F32)
            nc.sync.dma_start(out=ploc_t[:], in_=ploc_ext[:])
            iota_t = consts.tile([128, 128], F32)
            nc.sync.dma_start(out=iota_t[:], in_=iota_ext[:])
            q_t = consts.tile([128, D], F32)
            nc.gpsimd.dma_start(out=q_t[:], in_=q_ext[0:1, :].to_broadcast([128, D]))
            w1a_t = consts.tile([128, D], F32)
            nc.sync.dma_start(out=w1a_t[:], in_=w1_ext[0:128, :])
            w1b_t = consts.tile([128, D], F32)
            nc.sync.dma_start(out=w1b_t[:], in_=w1_ext[128:256, :])
            qb1_t = consts.tile([128, 2], F32)
            nc.sync.dma_start(out=qb1_t[:], in_=qb1_ext[:])
            w2_t = consts.tile([128, 2], F32)
            nc.sync.dma_start(out=w2_t[:], in_=w2_ext[:])
            b2_t = consts.tile([1, 1], F32)
            nc.sync.dma_start(out=b2_t[:], in_=b2_ext[:])
            eps_t = consts.tile([128, 1], F32)
            nc.vector.memset(eps_t[:], EPS)
            ident_t = consts.tile([128, 128], F32)
            make_identity(nc, ident_t[:])
            bc = {}
            for used, name, ext in (
                (use_gamma_e, "gamma_e", ge_ext),
                (use_beta_e, "beta_e", be_ext),
                (use_gamma_p, "gamma_p", gp_ext),
                (use_beta_p, "beta_p", bp_ext),
            ):
                if used:
                    t = consts.tile([128, D], F32, name=f"{name}_t")
                    nc.gpsimd.dma_start(
                        out=t[:], in_=ext[0:1, :].to_broadcast([128, D])
                    )
                    bc[name] = t

            g_table = dram.tile([ENT_PAD, D], BF16)
            psg_part_cs = [
                dram.tile([BLK_PER_CHUNK * BLK, D], F32, name=f"psg_part_{c}")
                for c in range(NCHUNK)
            ]
            rs_out_cs = [
                dram.tile([PSG_SH // NCHUNK, D], F32, name=f"rs_out_{c}")
                for c in range(NCHUNK)
            ]
            g_r = g_table[:].rearrange("(c p) d -> p c d", p=128)

            def layer_norm(x_ap, rows, gamma, beta):
                stats = ent_sc.tile([128, 6], F32, name="ln_stats")
                mv = ent_sc.tile([128, 2], F32, name="ln_mv")
                nc.vector.bn_stats(out=stats[:rows], in_=x_ap[:rows])
                nc.vector.bn_aggr(out=mv[:rows], in_=stats[:rows])
                nc.scalar.activation(
                    out=mv[:rows, 1:2],
                    in_=mv[:rows, 1:2],
                    func=mybir.ActivationFunctionType.Sqrt,
                    bias=eps_t[:rows],
                )
                nc.vector.reciprocal(out=mv[:rows, 1:2], in_=mv[:rows, 1:2])
                nc.vector.tensor_scalar(
                    out=x_ap[:rows],
                    in0=x_ap[:rows],
                    scalar1=mv[:rows, 0:1],
                    scalar2=mv[:rows, 1:2],
                    op0=mybir.AluOpType.subtract,
                    op1=mybir.AluOpType.mult,
                )
                if gamma is not None:
                    nc.vector.tensor_mul(x_ap[:rows], x_ap[:rows], gamma[:rows])
                if beta is not None:
                    nc.vector.tensor_add(x_ap[:rows], x_ap[:rows], beta[:rows])

            for _rep in range(reps):
                # ================= entity phase =================
                for c0 in range(0, ENT_TILES, ECHUNK) if phases[0] else ():
                    ntc = min(ECHUNK, ENT_TILES - c0)
                    hl_t = ent.tile([128, ECHUNK, D], F32, name="hl_t")
                    hg_t = ent.tile([128, ECHUNK, D], F32, name="hg_t")
                    g_t = ent.tile([128, ECHUNK, D], BF16, name="g_t")
                    nc.sync.dma_start(
                        out=hl_t[:, :ntc, :], in_=hl_r[:, c0 : c0 + ntc, :]
                    )
                    nc.sync.dma_start(
                        out=hg_t[:, :ntc, :], in_=hg_r[:, c0 : c0 + ntc, :]
                    )
                    for j in range(ntc):
                        a = hl_t[:, j, :]
                        b = hg_t[:, j, :]
                        nc.scalar.activation(
                            out=a, in_=a,
                            func=mybir.ActivationFunctionType.Copy,
                            scale=1.0 - ALPHA,
                        )
                        nc.gpsimd.tensor_scalar_mul(out=b, in0=b, scalar1=ALPHA)
                        nc.vector.tensor_add(a, a, b)
                        layer_norm(a, 128, bc.get("gamma_e"), bc.get("beta_e"))
                        xq = ent_sc.tile([128, D], F32, name="xq")
                        rel = ent_sc.tile([128, 1], F32, name="rel")
                        nc.vector.tensor_mul(xq[:], a, q_t[:])
                        nc.vector.reduce_sum(
                            out=rel[:], in_=xq[:], axis=mybir.AxisListType.X
                        )
                        nc.scalar.activation(
                            out=rel[:], in_=rel[:],
                            func=mybir.ActivationFunctionType.Sigmoid,
                        )
                        nc.scalar.activation(
                            out=g_t[:, j, :], in_=a,
                            func=mybir.ActivationFunctionType.Copy,
                            scale=rel[:, 0:1],
                        )
                    nc.sync.dma_start(
                        out=g_r[:, c0 : c0 + ntc, :], in_=g_t[:, :ntc, :]
                    )

                # ========== edge + chunked RS + chunk-pipelined MLP ==========
                score_sb = mlp.tile([1, PSG_PAD], F32, name="score_sb")
                nc.vector.memset(score_sb[:], 0.0)

                def mlp_tile(i):
                    r0 = i * 128
                    rows = min(128, PSG_SH - r0)
                    hx = mlp.tile([128, D], F32, name="hx")
                    cx = mlp.tile([128, D], F32, name="cx")
                    if rows < 128:
                        nc.vector.memset(cx[:], 0.0)
                    nc.scalar.dma_start(out=hx[:], in_=hp_ext[r0 : r0 + 128, :])
                    csz = PSG_SH // NCHUNK
                    c0_, c1_ = r0 // csz, (r0 + rows - 1) // csz
                    if c0_ == c1_:
                        nc.scalar.dma_start(
                            out=cx[:rows, :],
                            in_=rs_out_cs[c0_][r0 - c0_ * csz : r0 - c0_ * csz + rows, :],
                        )
                    else:
                        ra = (c0_ + 1) * csz - r0
                        nc.scalar.dma_start(
                            out=cx[:ra, :],
                            in_=rs_out_cs[c0_][r0 - c0_ * csz : r0 - c0_ * csz + ra, :],
                        )
                        nc.scalar.dma_start(
                            out=cx[ra:rows, :],
                            in_=rs_out_cs[c1_][0 : rows - ra, :],
                        )
                    nc.vector.tensor_add(hx[:], hx[:], cx[:])
                    layer_norm(hx[:], 128, bc.get("gamma_p"), bc.get("beta_p"))
                    xts = []
                    for dc in range(2):
                        tp = tp_ps.tile([128, 128], F32, space="PSUM", name="tp")
                        nc.tensor.transpose(
                            out=tp[:],
                            in_=hx[:, dc * 128 : (dc + 1) * 128],
                            identity=ident_t[:],
                        )
                        xt = mlp.tile([128, 128], F32, name="xt")
                        nc.vector.tensor_copy(out=xt[:], in_=tp[:])
                        xts.append(xt)
                    hids = []
                    for jc in range(2):
                        hp_ = hid_ps.tile([128, 128], F32, space="PSUM", name="hp_")
                        nc.tensor.matmul(
                            out=hp_[:],
                            lhsT=w1a_t[:, jc * 128 : (jc + 1) * 128],
                            rhs=xts[0][:],
                            start=True,
                            stop=False,
                        )
                        nc.tensor.matmul(
                            out=hp_[:],
                            lhsT=w1b_t[:, jc * 128 : (jc + 1) * 128],
                            rhs=xts[1][:],
                            start=False,
                            stop=True,
                        )
                        hid = mlp.tile([128, 128], F32, name="hid")
                        nc.scalar.activation(
                            out=hid[:],
                            in_=hp_[:],
                            func=mybir.ActivationFunctionType.Relu,
                            bias=qb1_t[:, jc : jc + 1],
                        )
                        hids.append(hid)
                    sc = sc_ps.tile([1, 128], F32, space="PSUM", name="sc")
                    nc.tensor.matmul(
                        out=sc[:], lhsT=w2_t[:, 0:1], rhs=hids[0][:],
                        start=True, stop=False,
                    )
                    nc.tensor.matmul(
                        out=sc[:], lhsT=w2_t[:, 1:2], rhs=hids[1][:],
                        start=False, stop=True,
                    )
                    if use_b2:
                        nc.vector.tensor_scalar(
                            out=score_sb[:, r0 : r0 + 128],
                            in0=sc[:],
                            scalar1=b2_t[0:1, 0:1],
                            scalar2=None,
                            op0=mybir.AluOpType.add,
                        )
                    else:
                        nc.vector.tensor_copy(
                            out=score_sb[:, r0 : r0 + 128], in_=sc[:]
                        )

                mlp_done = 0
                for ch in range(NCHUNK):
                    for s0 in (
                        range(
                            ch * BLK_PER_CHUNK, (ch + 1) * BLK_PER_CHUNK, GBLK
                        )
                        if phases[1]
                        else ()
                    ):
                        nidx = GBLK * t_b * 128
                        gat = gatp.tile([128, GBLK * t_b, D], BF16, name="gat")
                        c0 = s0 * t_b * 8  # idx16 col offset (16 idx/col)
                        nc.gpsimd.dma_gather(
                            out_ap=gat[:],
                            in_ap=g_table[:],
                            idxs_ap=idx_t[:, c0 : c0 + nidx // 16],
                            num_idxs=nidx,
                            num_idxs_reg=nidx,
                            elem_size=D,
                            queue_num=(s0 // GBLK) % NQ,
                        )
                        for bi in range(GBLK):
                            ordi = s0 + bi
                            acc = eps_ps.tile(
                                [128, D], F32, space="PSUM", name="acc"
                            )
                            for t in range(t_b):
                                col = ordi * t_b + t
                                oh = ohp.tile([128, BLK], BF16, name="oh")
                                nc.vector.tensor_tensor(
                                    out=oh[:],
                                    in0=ploc_t[:, col : col + 1].to_broadcast(
                                        [128, BLK]
                                    ),
                                    in1=iota_t[:, :BLK],
                                    op=mybir.AluOpType.is_equal,
                                )
                                nc.tensor.matmul(
                                    out=acc[:BLK, :],
                                    lhsT=oh[:],
                                    rhs=gat[:, bi * t_b + t, :],
                                    start=(t == 0),
                                    stop=(t == t_b - 1),
                                )
                            ev = evp.tile([128, D], F32, name="ev")
                            nc.vector.tensor_copy(out=ev[:BLK, :], in_=acc[:BLK, :])
                            lo = (ordi - ch * BLK_PER_CHUNK) * BLK
                            nc.scalar.dma_start(
                                out=psg_part_cs[ch][lo : lo + BLK, :],
                                in_=ev[:BLK, :],
                            )
                    if phases[2]:
                        nc.gpsimd.collective_compute(
                            "ReduceScatter",
                            mybir.AluOpType.add,
                            replica_groups=[list(range(NCORES))],
                            ins=[psg_part_cs[ch][:].opt()],
                            outs=[rs_out_cs[ch][:].opt()],
                        )
                    if phases[3]:
                        lim = PSG_SH // NCHUNK * (ch + 1)
                        while mlp_done < PSG_TILES and (
                            ch == NCHUNK - 1 or 128 * (mlp_done + 1) <= lim
                        ):
                            mlp_tile(mlp_done)
                            mlp_done += 1
                nc.sync.dma_start(out=out_ext[:], in_=score_sb[:])
    nc.compile()
    return nc


# ---------------------------------------------------------------------------
# host-side input prep
# ---------------------------------------------------------------------------
def pack_idx16(idx: np.ndarray) -> np.ndarray:
    n = idx.shape[0]
    arr = idx.astype(np.int16).reshape(n // 16, 16).T  # [16, n/16]
    return np.ascontiguousarray(np.tile(arr, (8, 1)))  # [128, n/16]


# block id B (= psg//BLK) -> processing/layout order ord: (chunk, rank, j)
_B = np.arange(NBLK)
_ORD_OF_BLK = (_B % 20 // 5) * 40 + (_B // 20) * 5 + (_B % 5)


def prep_in_maps(inputs: dict, t_b: int) -> list[dict]:
    ent_idx = np.asarray(inputs["ent_idx"])
    psg_idx = np.asarray(inputs["psg_idx"])
    hl = np.asarray(inputs["h_local_ent"])
    hg = np.asarray(inputs["h_ent_global"])
    hp = np.asarray(inputs["h_passage"])
    q = np.asarray(inputs["query_emb"]).reshape(1, D)
    w1 = np.asarray(inputs["W1"])
    b1 = np.asarray(inputs["b1"])
    w2 = np.asarray(inputs["W2"])
    b2 = np.asarray(inputs["b2"])

    qb1 = (q[0] @ w1[D:]) + b1  # [256]
    iota = np.ascontiguousarray(
        np.tile(np.arange(128, dtype=np.float32)[None, :], (128, 1))
    )
    n_slots = NBLK * t_b * 128
    tpb = t_b * 128  # slots per block

    in_maps = []
    for r in range(NCORES):
        lo = r * ENT_SH
        sel = np.nonzero((ent_idx >= lo) & (ent_idx < lo + ENT_SH))[0]
        e_loc = (ent_idx[sel] - lo).astype(np.int32)
        p_glob = psg_idx[sel].astype(np.int64)
        blocks = p_glob // BLK
        ordv = _ORD_OF_BLK[blocks]
        order = np.argsort(ordv, kind="stable")
        e_loc = e_loc[order]
        p_glob = p_glob[order]
        blk_s = blocks[order]
        ordv = ordv[order]
        counts = np.bincount(ordv, minlength=NBLK)
        assert counts.max() <= tpb, (counts.max(), tpb)
        starts = np.zeros(NBLK, np.int64)
        np.cumsum(counts[:-1], out=starts[1:])
        rank = np.arange(len(ordv)) - starts[ordv]
        pos = ordv * tpb + rank
        slots_e = np.zeros(n_slots, np.int16)
        slots_p = np.full(n_slots, 300.0, np.float32)
        slots_e[pos] = e_loc
        slots_p[pos] = (p_glob - blk_s * BLK).astype(np.float32)

        hl_sh = np.zeros((ENT_PAD, D), np.float32)
        hl_sh[:ENT_SH] = hl[lo : lo + ENT_SH]
        hg_sh = np.zeros((ENT_PAD, D), np.float32)
        hg_sh[:ENT_SH] = hg[lo : lo + ENT_SH]
        hp_sh = np.zeros((PSG_PAD, D), np.float32)
        hp_sh[:PSG_SH] = hp[r * PSG_SH : (r + 1) * PSG_SH]

        in_maps.append(
            {
                "hl": hl_sh,
                "hg": hg_sh,
                "hp": hp_sh,
                "q": q.astype(np.float32),
                "idx16": pack_idx16(slots_e),
                "ploc": np.ascontiguousarray(
                    slots_p.reshape(n_slots // 128, 128).T
                ),
                "iota": iota,
                "w1": np.ascontiguousarray(w1[:D]),
                "qb1": np.ascontiguousarray(qb1.reshape(2, 128).T),
                "w2": np.ascontiguousarray(w2[:, 0].reshape(2, 128).T),
                "b2": b2.reshape(1, 1).astype(np.float32),
                "gamma_e": np.asarray(inputs["gamma_e"]).reshape(1, D),
                "beta_e": np.asarray(inputs["beta_e"]).reshape(1, D),
                "gamma_p": np.asarray(inputs["gamma_p"]).reshape(1, D),
                "beta_p": np.asarray(inputs["beta_p"]).reshape(1, D),
            }
        )
    return in_maps


def _flags(inputs):
    return (
        not np.all(np.asarray(inputs["gamma_e"]) == 1.0),
        not np.all(np.asarray(inputs["beta_e"]) == 0.0),
        not np.all(np.asarray(inputs["gamma_p"]) == 1.0),
        not np.all(np.asarray(inputs["beta_p"]) == 0.0),
        not np.all(np.asarray(inputs["b2"]) == 0.0),
    )


def _pick_t_b(inputs):
    ent_idx = np.asarray(inputs["ent_idx"])
    psg_idx = np.asarray(inputs["psg_idx"])
    mx = 0
    for r in range(NCORES):
        lo = r * ENT_SH
        m = (ent_idx >= lo) & (ent_idx < lo + ENT_SH)
        cnt = np.bincount(psg_idx[m] // BLK, minlength=NBLK)
        mx = max(mx, int(cnt.max()))
    return max(4, (mx + 127) // 128)


# ---------------------------------------------------------------------------
# PJRT SPMD execution (axon)
# ---------------------------------------------------------------------------
class CompiledSpmd:
    def __init__(self, nc, n_cores: int):
        import jax
        from jax.experimental.shard_map import shard_map
        from jax.sharding import Mesh, NamedSharding, PartitionSpec

        from concourse.bass2jax import (
            _bass_exec_p,
            install_neuronx_cc_hook,
            partition_id_tensor,
        )

        self.jax = jax
        install_neuronx_cc_hook()
        self.n_cores = n_cores
        partition_name = (
            nc.partition_id_tensor.name if nc.partition_id_tensor else None
        )
        in_names, out_names, out_avals, zero_outs = [], [], [], []
        for alloc in nc.m.functions[0].allocations:
            if not isinstance(alloc, mybir.MemoryLocationSet):
                continue
            name = alloc.memorylocations[0].name
            if alloc.kind == "ExternalInput":
                if name != partition_name:
                    in_names.append(name)
            elif alloc.kind == "ExternalOutput":
                out_names.append(name)
                shape = tuple(alloc.tensor_shape)
                dtype = mybir.dt.np(alloc.dtype)
                out_avals.append(jax.core.ShapedArray(shape, dtype))
                zero_outs.append(np.zeros(shape, dtype))
        self.in_names = in_names
        self.out_names = out_names
        self.out_avals = out_avals
        self.zero_outs = zero_outs
        n_params = len(in_names)
        n_outs = len(out_avals)
        all_in_names = in_names + out_names
        if partition_name is not None:
            all_in_names.append(partition_name)
        donate = tuple(range(n_params, n_params + n_outs))

        def _body(*args):
            operands = list(args)
            if partition_name is not None:
                operands.append(partition_id_tensor())
            outs = _bass_exec_p.bind(
                *operands,
                out_avals=tuple(out_avals),
                in_names=tuple(all_in_names),
                out_names=tuple(out_names),
                lowering_input_output_aliases=(),
                sim_require_finite=True,
                sim_require_nnan=True,
                nc=nc,
            )
            return tuple(outs)

        devices = jax.devices()[:n_cores]
        assert len(devices) == n_cores
        self.mesh = Mesh(np.asarray(devices), ("core",))
        in_specs = (PartitionSpec("core"),) * (n_params + n_outs)
        out_specs = (PartitionSpec("core"),) * len(out_names)
        self.sharding = NamedSharding(self.mesh, PartitionSpec("core"))
        self.fn = jax.jit(
            shard_map(
                _body,
                mesh=self.mesh,
                in_specs=in_specs,
                out_specs=out_specs,
                check_rep=False,
            ),
            donate_argnums=donate,
            keep_unused=True,
        )
        self._resident = None

    def stage_inputs(self, in_maps):
        n = self.n_cores
        concat_in = [
            np.ascontiguousarray(
                np.concatenate(
                    [np.asarray(in_maps[c][k]) for c in range(n)], axis=0
                )
            )
            for k in self.in_names
        ]
        self._resident = [
            self.jax.device_put(x, self.sharding) for x in concat_in
        ]
        self.jax.block_until_ready(self._resident)

    def _zeros(self):
        n = self.n_cores
        return [
            self.jax.device_put(
                np.zeros((n * z.shape[0], *z.shape[1:]), z.dtype), self.sharding
            )
            for z in self.zero_outs
        ]

    def run(self):
        outs = self.fn(*self._resident, *self._zeros())
        self.jax.block_until_ready(outs)
        n = self.n_cores
        return [
            {
                k: np.asarray(outs[i]).reshape(n, *self.out_avals[i].shape)[c]
                for i, k in enumerate(self.out_names)
            }
            for c in range(n)
        ]

    def time_s(self, reps=20, warmup=3):
        import time

        for _ in range(warmup):
            self.jax.block_until_ready(self.fn(*self._resident, *self._zeros()))
        times = []
        for _ in range(reps):
            z = self._zeros()
            t0 = time.perf_counter()
            out = self.fn(*self._resident, *z)
            self.jax.block_until_ready(out)
            times.append(time.perf_counter() - t0)
        return float(np.min(times))


_CACHE = {}


def get_compiled(inputs, reps=1):
    t_b = _pick_t_b(inputs)
    flags = _flags(inputs)
    key = (reps, t_b, flags)
    if key not in _CACHE:
        nc = build_nc(reps, t_b, *flags)
        _CACHE[key] = (CompiledSpmd(nc, NCORES), t_b)
    return _CACHE[key]


def kernel(**inputs) -> np.ndarray:
    comp, t_b = get_compiled(inputs, reps=1)
    in_maps = prep_in_maps(inputs, t_b)
    comp.stage_inputs(in_maps)
    res = comp.run()
    out = np.concatenate([res[c]["out"][0, :PSG_SH] for c in range(NCORES)])
    return out.astype(np.float32)



# revision 42
# speedup vs baseline: 1.7755x; 1.7755x over previous
"""Trainium2 Bass kernel for AuditableHybridGNN (gnn_message_passing).

Computation (reference):
  h_ent = LN((1-a)*h_local + a*h_global) * gamma_e + beta_e        [100000,256]
  rel   = sigmoid(sum(h_ent * q, -1))                              [100000]
  ctx   = segment_sum(h_ent[ent_idx] * rel[ent_idx], psg_idx)      [20000,256]
  h_psg = LN(h_passage + ctx) * gamma_p + beta_p                   [20000,256]
  out   = relu([h_psg, q] @ W1 + b1) @ W2 + b2                     [20000]

Distribution over 8 NeuronCores (SPMD, one program):
  - entities sharded 12500/core: each core computes g = h_ent*rel (bf16) for
    its shard, in 4 row-quarters; after each quarter an AllGather ships it to
    every core (4 chunked AGs overlap the entity phase);
  - edges sharded by PASSAGE owner: each core owns 2500 passages (20 blocks
    of 125) and consumes every edge pointing at them.  Edges are bucketed by
    (entity-quarter k, local block b); the gathered table for quarter k is
    the AG output slab (8*quarter rows, int16-indexable).  Per (k,b): one
    dma_gather of t_q*128 edge rows, then a one-hot [edge x 125] matmul
    accumulating into PSUM; k-partials are summed into an SBUF ctx buffer.
    No ReduceScatter and no DRAM round-trip for the segment sums.
  - per block: LN(h_passage + ctx) + scoring MLP -> 125 scores; host
    concatenates the per-core [2500] outputs.
"""
import contextlib
import os
import sys

sys.path.insert(0, "/opt/trn_rl_repo")

import ml_dtypes
import numpy as np

import concourse.bass as bass
from concourse import bacc, mybir, tile
from concourse.masks import make_identity

F32 = mybir.dt.float32
BF16 = mybir.dt.bfloat16
I16 = mybir.dt.int16

NCORES = 8
N_ENT = 100000
N_PSG = 20000
N_EDGE = 500000
D = 256
ALPHA = 0.1
EPS = 1e-5

ENT_SH = N_ENT // NCORES  # 12500
PSG_SH = N_PSG // NCORES  # 2500
ENT_TILES = (ENT_SH + 127) // 128  # 98
ENT_PAD = ENT_TILES * 128  # 12544
BLK = 125
NBLK_L = PSG_SH // BLK  # 20 local blocks
NQUART = 4
QT = [25, 25, 24, 24]  # entity tiles per quarter (sum = 98)
Q_ROWS = [t * 128 for t in QT]  # [3200, 3200, 3072, 3072]
Q_START = [0, 3200, 6400, 9472]
SLAB_ROWS = [NCORES * r for r in Q_ROWS]  # max 25600 < int16 range
NQ = int(os.environ.get("BASS_NQ", "4"))  # SWDGE queues for gathers
ECHUNK = 8  # entity tiles per DMA batch (also Sigmoid act-table batch size)
GMAX = 8  # max tiles per dma_gather (1024-idx SWDGE ring)


def build_nc(reps, t_q, use_gamma_e, use_beta_e, use_gamma_p, use_beta_p, use_b2,
             phases=(1, 1, 1, 1)):
    """phases = (ent, edge, ag, mlp) repeat counts (for differential timing)."""
    nc = bacc.Bacc(
        "TRN2",
        target_bir_lowering=False,
        debug=False,
        num_devices=NCORES,
        num_swdge_queues=NQ,
    )
    n_gi = NQUART * NBLK_L  # 80 gather groups
    n_slots = n_gi * t_q * 128

    hl_ext = nc.dram_tensor("hl", [ENT_PAD, D], F32, kind="ExternalInput")
    hg_ext = nc.dram_tensor("hg", [ENT_PAD, D], F32, kind="ExternalInput")
    hp_ext = nc.dram_tensor("hp", [PSG_SH, D], F32, kind="ExternalInput")
    q_ext = nc.dram_tensor("q", [1, D], F32, kind="ExternalInput")
    idx_ext = nc.dram_tensor("idx16", [128, n_slots // 16], I16, kind="ExternalInput")
    ploc_ext = nc.dram_tensor("ploc", [128, n_slots // 128], BF16, kind="ExternalInput")
    iota_ext = nc.dram_tensor("iota", [128, 128], BF16, kind="ExternalInput")
    w1_ext = nc.dram_tensor("w1", [D, D], F32, kind="ExternalInput")
    qb1_ext = nc.dram_tensor("qb1", [128, 2], F32, kind="ExternalInput")
    w2_ext = nc.dram_tensor("w2", [128, 2], F32, kind="ExternalInput")
    b2_ext = nc.dram_tensor("b2", [1, 1], F32, kind="ExternalInput")
    ge_ext = nc.dram_tensor("gamma_e", [1, D], F32, kind="ExternalInput")
    be_ext = nc.dram_tensor("beta_e", [1, D], F32, kind="ExternalInput")
    gp_ext = nc.dram_tensor("gamma_p", [1, D], F32, kind="ExternalInput")
    bp_ext = nc.dram_tensor("beta_p", [1, D], F32, kind="ExternalInput")
    out_ext = nc.dram_tensor("out", [1, PSG_SH], F32, kind="ExternalOutput")

    hl_r = hl_ext[:].rearrange("(c p) d -> p c d", p=128)
    hg_r = hg_ext[:].rearrange("(c p) d -> p c d", p=128)

    with tile.TileContext(nc) as tc:
        with (
            tc.tile_pool(name="consts", bufs=1) as consts,
            tc.tile_pool(name="ent", bufs=3) as ent,
            tc.tile_pool(name="ent_sc", bufs=4) as ent_sc,
            tc.tile_pool(name="gatp", bufs=20) as gatp,
            tc.tile_pool(name="ohp", bufs=6) as ohp,
            tc.tile_pool(name="ctxp", bufs=1) as ctxp,
            tc.tile_pool(name="mlp", bufs=3) as mlp,
            tc.tile_pool(name="eps_ps", bufs=3, space="PSUM") as eps_ps,
            tc.tile_pool(name="tp_ps", bufs=2, space="PSUM") as tp_ps,
            tc.tile_pool(name="hid_ps", bufs=2, space="PSUM") as hid_ps,
            tc.tile_pool(name="sc_ps", bufs=1, space="PSUM") as sc_ps,
            tc.tile_pool(name="dram", bufs=1, space="DRAM") as dram,
        ):
            # ---- constants loaded once ----
            idx_t = consts.tile([128, n_slots // 16], I16)
            nc.sync.dma_start(out=idx_t[:], in_=idx_ext[:])
            ploc_t = consts.tile([128, n_slots // 128], BF16)
            nc.sync.dma_start(out=ploc_t[:], in_=ploc_ext[:])
            iota_t = consts.tile([128, 128], BF16)
            nc.sync.dma_start(out=iota_t[:], in_=iota_ext[:])
            q_t = consts.tile([128, D], F32)
            nc.sync.dma_start(out=q_t[:], in_=q_ext[0:1, :].to_broadcast([128, D]))
            w1a_t = consts.tile([128, D], F32)
            nc.sync.dma_start(out=w1a_t[:], in_=w1_ext[0:128, :])
            w1b_t = consts.tile([128, D], F32)
            nc.sync.dma_start(out=w1b_t[:], in_=w1_ext[128:256, :])
            qb1_t = consts.tile([128, 2], F32)
            nc.sync.dma_start(out=qb1_t[:], in_=qb1_ext[:])
            w2_t = consts.tile([128, 2], F32)
            nc.sync.dma_start(out=w2_t[:], in_=w2_ext[:])
            b2_t = consts.tile([1, 1], F32)
            nc.sync.dma_start(out=b2_t[:], in_=b2_ext[:])
            eps_t = consts.tile([128, 1], F32)
            nc.vector.memset(eps_t[:], EPS)
            ident_t = consts.tile([128, 128], F32)
            make_identity(nc, ident_t[:])
            bc = {}
            for used, name, ext in (
                (use_gamma_e, "gamma_e", ge_ext),
                (use_beta_e, "beta_e", be_ext),
                (use_gamma_p, "gamma_p", gp_ext),
                (use_beta_p, "beta_p", bp_ext),
            ):
                if used:
                    t = consts.tile([128, D], F32, name=f"{name}_t")
                    nc.sync.dma_start(
                        out=t[:], in_=ext[0:1, :].to_broadcast([128, D])
                    )
                    bc[name] = t

            g_qs = [
                dram.tile([Q_ROWS[k], D], BF16, name=f"g_q{k}") for k in range(NQUART)
            ]
            g_q_rs = [
                g_qs[k][:].rearrange("(c p) d -> p c d", p=128) for k in range(NQUART)
            ]
            slabs = [
                dram.tile([SLAB_ROWS[k], D], BF16, name=f"slab{k}")
                for k in range(NQUART)
            ]

            use_scopes = bool(os.environ.get("BASS_SCOPES"))

            @contextlib.contextmanager
            def scope(name):
                if use_scopes:
                    with nc.named_scope(name):
                        yield
                else:
                    yield

            def s_enter(name):
                return nc.enter_named_scope(name, False)[0] if use_scopes else None

            def s_leave(name, sid):
                if sid is not None:
                    nc.leave_named_scope(name, sid, False)

            def layer_norm(x_ap, rows, gamma, beta):
                stats = ent_sc.tile([128, 6], F32, name="ln_stats")
                mv = ent_sc.tile([128, 2], F32, name="ln_mv")
                nc.vector.bn_stats(out=stats[:rows], in_=x_ap[:rows])
                nc.vector.bn_aggr(out=mv[:rows], in_=stats[:rows])
                nc.scalar.activation(
                    out=mv[:rows, 1:2],
                    in_=mv[:rows, 1:2],
                    func=mybir.ActivationFunctionType.Sqrt,
                    bias=eps_t[:rows],
                )
                nc.vector.reciprocal(out=mv[:rows, 1:2], in_=mv[:rows, 1:2])
                nc.vector.tensor_scalar(
                    out=x_ap[:rows],
                    in0=x_ap[:rows],
                    scalar1=mv[:rows, 0:1],
                    scalar2=mv[:rows, 1:2],
                    op0=mybir.AluOpType.subtract,
                    op1=mybir.AluOpType.mult,
                )
                if gamma is not None:
                    nc.vector.tensor_mul(x_ap[:rows], x_ap[:rows], gamma[:rows])
                if beta is not None:
                    nc.vector.tensor_add(x_ap[:rows], x_ap[:rows], beta[:rows])

            # tile index -> (quarter, tile offset within quarter)
            def quart_of(tile_idx):
                for k in range(NQUART):
                    if tile_idx < sum(QT[: k + 1]):
                        return k, tile_idx - sum(QT[:k])
                raise AssertionError

            def emit_ag(k):
                for _ in range(phases[2]):
                    with scope("ag"):
                        nc.gpsimd.collective_compute(
                            "AllGather",
                            mybir.AluOpType.bypass,
                            replica_groups=[list(range(NCORES))],
                            ins=[g_qs[k][:].opt()],
                            outs=[slabs[k][:].opt()],
                        )

            for _rep in range(reps):
                # ================= entity phase + chunked AllGathers ========
                gats = {}

                def emit_gathers(k):
                    for b in range(NBLK_L):
                        gi = k * NBLK_L + b
                        gat = gatp.tile([128, t_q, D], BF16, name="gat")
                        for t0 in range(0, t_q, GMAX):
                            nt = min(GMAX, t_q - t0)
                            c0 = (gi * t_q + t0) * 8  # idx16 col offset (16/col)
                            nc.gpsimd.dma_gather(
                                out_ap=gat[:, t0 : t0 + nt, :],
                                in_ap=slabs[k][:],
                                idxs_ap=idx_t[:, c0 : c0 + nt * 8],
                                num_idxs=nt * 128,
                                num_idxs_reg=nt * 128,
                                elem_size=D,
                                queue_num=gi % NQ,
                            )
                        gats[(k, b)] = gat

                _se = s_enter("ent")
                ag_done = 0
                ent_iters = [
                    c for _ in range(phases[0]) for c in range(0, ENT_TILES, ECHUNK)
                ]
                for c0 in ent_iters:
                    ntc = min(ECHUNK, ENT_TILES - c0)
                    hl_t = ent.tile([128, ECHUNK, D], F32, name="hl_t")
                    hg_t = ent.tile([128, ECHUNK, D], F32, name="hg_t")
                    g_t = ent.tile([128, ECHUNK, D], BF16, name="g_t")
                    nc.sync.dma_start(
                        out=hl_t[:, :ntc, :], in_=hl_r[:, c0 : c0 + ntc, :]
                    )
                    nc.sync.dma_start(
                        out=hg_t[:, :ntc, :], in_=hg_r[:, c0 : c0 + ntc, :]
                    )
                    relc = ent_sc.tile([128, ECHUNK], F32, name="relc")
                    for j in range(ntc):
                        a = hl_t[:, j, :]
                        b = hg_t[:, j, :]
                        # LN is scale-invariant: LN(.9*hl+.1*hg) == LN(hl+(1/9)*hg)
                        nc.scalar.activation(
                            out=b, in_=b,
                            func=mybir.ActivationFunctionType.Copy,
                            scale=ALPHA / (1.0 - ALPHA),
                        )
                        nc.vector.tensor_add(a, a, b)
                        layer_norm(a, 128, bc.get("gamma_e"), bc.get("beta_e"))
                        xq = ent_sc.tile([128, D], F32, name="xq")
                        nc.vector.tensor_mul(xq[:], a, q_t[:])
                        nc.vector.reduce_sum(
                            out=relc[:, j : j + 1],
                            in_=xq[:],
                            axis=mybir.AxisListType.X,
                        )
                    nc.scalar.activation(
                        out=relc[:, :ntc],
                        in_=relc[:, :ntc],
                        func=mybir.ActivationFunctionType.Sigmoid,
                    )
                    for j in range(ntc):
                        nc.scalar.activation(
                            out=g_t[:, j, :],
                            in_=hl_t[:, j, :],
                            func=mybir.ActivationFunctionType.Copy,
                            scale=relc[:, j : j + 1],
                        )
                    # write g tiles to their quarter tensors (may straddle)
                    j0 = 0
                    while j0 < ntc:
                        k, off = quart_of((c0 + j0) % ENT_TILES)
                        nsp = min(ntc - j0, QT[k] - off)
                        nc.sync.dma_start(
                            out=g_q_rs[k][:, off : off + nsp, :],
                            in_=g_t[:, j0 : j0 + nsp, :],
                        )
                        j0 += nsp
                    # fire AG for completed quarters
                    last_tile = (c0 % ENT_TILES) + ntc
                    while ag_done < NQUART and last_tile >= sum(QT[: ag_done + 1]):
                        emit_ag(ag_done)
                        if phases[1]:
                            emit_gathers(ag_done)
                        ag_done += 1
                if not phases[0]:
                    for k in range(NQUART):
                        emit_ag(k)
                        if phases[1]:
                            emit_gathers(k)
                s_leave("ent", _se)

                # ============ edge phase: gather + one-hot matmul ===========
                ctx_sb = ctxp.tile([128, NBLK_L, D], F32, name="ctx_sb")
                if not phases[1]:  # timing configs only: keep ctx_sb readable
                    nc.vector.memset(ctx_sb[:], 0.0)
                score_sb = mlp.tile([1, PSG_SH], F32, name="score_sb")

                # ================= per-block LN + scoring MLP ===============
                def mlp_blk(b):
                    _sm = s_enter("mlp")
                    hx = mlp.tile([128, D], F32, name="hx")
                    nc.scalar.dma_start(
                        out=hx[:BLK, :], in_=hp_ext[b * BLK : (b + 1) * BLK, :]
                    )
                    nc.vector.tensor_add(hx[:BLK, :], hx[:BLK, :], ctx_sb[:BLK, b, :])
                    layer_norm(hx, BLK, bc.get("gamma_p"), bc.get("beta_p"))
                    xts = []
                    for dc in range(2):
                        tp = tp_ps.tile([128, 128], F32, space="PSUM", name="tp")
                        nc.tensor.transpose(
                            out=tp[:, :BLK],
                            in_=hx[:BLK, dc * 128 : (dc + 1) * 128],
                            identity=ident_t[:BLK, :BLK],
                        )
                        xt = mlp.tile([128, 128], F32, name="xt")
                        nc.scalar.copy(out=xt[:, :BLK], in_=tp[:, :BLK])
                        xts.append(xt)
                    hids = []
                    for jc in range(2):
                        hp_ = hid_ps.tile([128, 128], F32, space="PSUM", name="hp_")
                        nc.tensor.matmul(
                            out=hp_[:, :BLK],
                            lhsT=w1a_t[:, jc * 128 : (jc + 1) * 128],
                            rhs=xts[0][:, :BLK],
                            start=True,
                            stop=False,
                        )
                        nc.tensor.matmul(
                            out=hp_[:, :BLK],
                            lhsT=w1b_t[:, jc * 128 : (jc + 1) * 128],
                            rhs=xts[1][:, :BLK],
                            start=False,
                            stop=True,
                        )
                        hid = mlp.tile([128, 128], F32, name="hid")
                        nc.scalar.activation(
                            out=hid[:, :BLK],
                            in_=hp_[:, :BLK],
                            func=mybir.ActivationFunctionType.Relu,
                            bias=qb1_t[:, jc : jc + 1],
                        )
                        hids.append(hid)
                    sc = sc_ps.tile([1, 128], F32, space="PSUM", name="sc")
                    nc.tensor.matmul(
                        out=sc[:, :BLK], lhsT=w2_t[:, 0:1], rhs=hids[0][:, :BLK],
                        start=True, stop=False,
                    )
                    nc.tensor.matmul(
                        out=sc[:, :BLK], lhsT=w2_t[:, 1:2], rhs=hids[1][:, :BLK],
                        start=False, stop=True,
                    )
                    if use_b2:
                        nc.vector.tensor_scalar(
                            out=score_sb[:, b * BLK : (b + 1) * BLK],
                            in0=sc[:, :BLK],
                            scalar1=b2_t[0:1, 0:1],
                            scalar2=None,
                            op0=mybir.AluOpType.add,
                        )
                    else:
                        nc.vector.tensor_copy(
                            out=score_sb[:, b * BLK : (b + 1) * BLK], in_=sc[:, :BLK]
                        )
                    s_leave("mlp", _sm)


                edge_iters = [
                    (k, b)
                    for _ in range(phases[1])
                    for k in range(NQUART)
                    for b in range(NBLK_L)
                ]
                _sg = s_enter("edge")
                for k, b in edge_iters:
                    gi = k * NBLK_L + b
                    gat = gats[(k, b)]
                    acc = eps_ps.tile([128, D], F32, space="PSUM", name="acc")
                    for t in range(t_q):
                        col = gi * t_q + t
                        oh = ohp.tile([128, BLK], BF16, name="oh")
                        nc.vector.tensor_tensor(
                            out=oh[:],
                            in0=ploc_t[:, col : col + 1].to_broadcast([128, BLK]),
                            in1=iota_t[:, :BLK],
                            op=mybir.AluOpType.is_equal,
                        )
                        nc.tensor.matmul(
                            out=acc[:BLK, :],
                            lhsT=oh[:],
                            rhs=gat[:, t, :],
                            start=(t == 0),
                            stop=(t == t_q - 1),
                        )
                    if k == 0:
                        nc.scalar.copy(out=ctx_sb[:BLK, b, :], in_=acc[:BLK, :])
                    else:
                        nc.vector.tensor_add(
                            ctx_sb[:BLK, b, :], ctx_sb[:BLK, b, :], acc[:BLK, :]
                        )
                    if k == NQUART - 1 and phases[3]:
                        for _ in range(phases[3]):
                            mlp_blk(b)
                s_leave("edge", _sg)

                if not phases[3]:  # timing configs only
                    nc.vector.memset(score_sb[:], 0.0)
                if phases[3] and not phases[1]:
                    for _ in range(phases[3]):
                        for b in range(NBLK_L):
                            mlp_blk(b)
                nc.sync.dma_start(out=out_ext[:], in_=score_sb[:])
    nc.compile()
    return nc


# ---------------------------------------------------------------------------
# host-side input prep
# ---------------------------------------------------------------------------
def pack_idx16(idx: np.ndarray) -> np.ndarray:
    n = idx.shape[0]
    arr = idx.astype(np.int16).reshape(n // 16, 16).T  # [16, n/16]
    return np.ascontiguousarray(np.tile(arr, (8, 1)))  # [128, n/16]


def _edge_keys(ent_idx, psg_idx, r):
    """Return (sel, gi, slabrow, pin) for core r's edges."""
    lo = r * PSG_SH
    sel = np.nonzero((psg_idx >= lo) & (psg_idx < lo + PSG_SH))[0]
    p = (psg_idx[sel] - lo).astype(np.int64)
    e = ent_idx[sel].astype(np.int64)
    b = p // BLK
    pin = p % BLK
    owner = e // ENT_SH
    l = e % ENT_SH
    qk = np.searchsorted(np.asarray(Q_START[1:]), l, side="right")
    qrows = np.asarray(Q_ROWS)[qk]
    qstart = np.asarray(Q_START)[qk]
    slabrow = owner * qrows + (l - qstart)
    gi = qk * NBLK_L + b
    return sel, gi, slabrow, pin


def _pick_t_q(inputs):
    ent_idx = np.asarray(inputs["ent_idx"])
    psg_idx = np.asarray(inputs["psg_idx"])
    mx = 0
    for r in range(NCORES):
        _, gi, _, _ = _edge_keys(ent_idx, psg_idx, r)
        cnt = np.bincount(gi, minlength=NQUART * NBLK_L)
        mx = max(mx, int(cnt.max()))
    return (mx + 127) // 128


def prep_in_maps(inputs: dict, t_q: int) -> list[dict]:
    ent_idx = np.asarray(inputs["ent_idx"])
    psg_idx = np.asarray(inputs["psg_idx"])
    hl = np.asarray(inputs["h_local_ent"])
    hg = np.asarray(inputs["h_ent_global"])
    hp = np.asarray(inputs["h_passage"])
    q = np.asarray(inputs["query_emb"]).reshape(1, D)
    w1 = np.asarray(inputs["W1"])
    b1 = np.asarray(inputs["b1"])
    w2 = np.asarray(inputs["W2"])
    b2 = np.asarray(inputs["b2"])

    qb1 = (q[0] @ w1[D:]) + b1  # [256]
    iota = np.ascontiguousarray(
        np.tile(np.arange(128, dtype=np.float32)[None, :], (128, 1))
    ).astype(ml_dtypes.bfloat16)
    n_gi = NQUART * NBLK_L
    n_slots = n_gi * t_q * 128
    tpg = t_q * 128

    in_maps = []
    for r in range(NCORES):
        _, gi, slabrow, pin = _edge_keys(ent_idx, psg_idx, r)
        order = np.argsort(gi, kind="stable")
        gi = gi[order]
        slabrow = slabrow[order]
        pin = pin[order]
        counts = np.bincount(gi, minlength=n_gi)
        assert counts.max() <= tpg, (counts.max(), tpg)
        starts = np.zeros(n_gi, np.int64)
        np.cumsum(counts[:-1], out=starts[1:])
        rank = np.arange(len(gi)) - starts[gi]
        pos = gi * tpg + rank
        slots_e = np.zeros(n_slots, np.int16)
        slots_p = np.full(n_slots, 300.0, np.float32)
        slots_e[pos] = slabrow.astype(np.int16)
        slots_p[pos] = pin.astype(np.float32)

        lo = r * ENT_SH
        hl_sh = np.zeros((ENT_PAD, D), np.float32)
        hl_sh[:ENT_SH] = hl[lo : lo + ENT_SH]
        hg_sh = np.zeros((ENT_PAD, D), np.float32)
        hg_sh[:ENT_SH] = hg[lo : lo + ENT_SH]
        hp_sh = np.ascontiguousarray(hp[r * PSG_SH : (r + 1) * PSG_SH])

        in_maps.append(
            {
                "hl": hl_sh,
                "hg": hg_sh,
                "hp": hp_sh,
                "q": q.astype(np.float32),
                "idx16": pack_idx16(slots_e),
                "ploc": np.ascontiguousarray(
                    slots_p.reshape(n_slots // 128, 128).T
                ).astype(ml_dtypes.bfloat16),
                "iota": iota,
                "w1": np.ascontiguousarray(w1[:D]),
                "qb1": np.ascontiguousarray(qb1.reshape(2, 128).T),
                "w2": np.ascontiguousarray(w2[:, 0].reshape(2, 128).T),
                "b2": b2.reshape(1, 1).astype(np.float32),
                "gamma_e": np.asarray(inputs["gamma_e"]).reshape(1, D),
                "beta_e": np.asarray(inputs["beta_e"]).reshape(1, D),
                "gamma_p": np.asarray(inputs["gamma_p"]).reshape(1, D),
                "beta_p": np.asarray(inputs["beta_p"]).reshape(1, D),
            }
        )
    return in_maps


def _flags(inputs):
    return (
        not np.all(np.asarray(inputs["gamma_e"]) == 1.0),
        not np.all(np.asarray(inputs["beta_e"]) == 0.0),
        not np.all(np.asarray(inputs["gamma_p"]) == 1.0),
        not np.all(np.asarray(inputs["beta_p"]) == 0.0),
        not np.all(np.asarray(inputs["b2"]) == 0.0),
    )


# backwards-compat alias used by test.py
_pick_t_b = _pick_t_q


# ---------------------------------------------------------------------------
# PJRT SPMD execution (axon)
# ---------------------------------------------------------------------------
class CompiledSpmd:
    def __init__(self, nc, n_cores: int):
        import jax
        from jax.experimental.shard_map import shard_map
        from jax.sharding import Mesh, NamedSharding, PartitionSpec

        from concourse.bass2jax import (
            _bass_exec_p,
            install_neuronx_cc_hook,
            partition_id_tensor,
        )

        self.jax = jax
        install_neuronx_cc_hook()
        self.n_cores = n_cores
        partition_name = (
            nc.partition_id_tensor.name if nc.partition_id_tensor else None
        )
        in_names, out_names, out_avals, zero_outs = [], [], [], []
        for alloc in nc.m.functions[0].allocations:
            if not isinstance(alloc, mybir.MemoryLocationSet):
                continue
            name = alloc.memorylocations[0].name
            if alloc.kind == "ExternalInput":
                if name != partition_name:
                    in_names.append(name)
            elif alloc.kind == "ExternalOutput":
                out_names.append(name)
                shape = tuple(alloc.tensor_shape)
                dtype = mybir.dt.np(alloc.dtype)
                out_avals.append(jax.core.ShapedArray(shape, dtype))
                zero_outs.append(np.zeros(shape, dtype))
        self.in_names = in_names
        self.out_names = out_names
        self.out_avals = out_avals
        self.zero_outs = zero_outs
        n_params = len(in_names)
        n_outs = len(out_avals)
        all_in_names = in_names + out_names
        if partition_name is not None:
            all_in_names.append(partition_name)
        donate = tuple(range(n_params, n_params + n_outs))

        def _body(*args):
            operands = list(args)
            if partition_name is not None:
                operands.append(partition_id_tensor())
            outs = _bass_exec_p.bind(
                *operands,
                out_avals=tuple(out_avals),
                in_names=tuple(all_in_names),
                out_names=tuple(out_names),
                lowering_input_output_aliases=(),
                sim_require_finite=True,
                sim_require_nnan=True,
                nc=nc,
            )
            return tuple(outs)

        devices = jax.devices()[:n_cores]
        assert len(devices) == n_cores
        self.mesh = Mesh(np.asarray(devices), ("core",))
        in_specs = (PartitionSpec("core"),) * (n_params + n_outs)
        out_specs = (PartitionSpec("core"),) * len(out_names)
        self.sharding = NamedSharding(self.mesh, PartitionSpec("core"))
        self.fn = jax.jit(
            shard_map(
                _body,
                mesh=self.mesh,
                in_specs=in_specs,
                out_specs=out_specs,
                check_rep=False,
            ),
            donate_argnums=donate,
            keep_unused=True,
        )
        self._resident = None

    def stage_inputs(self, in_maps):
        n = self.n_cores
        concat_in = [
            np.ascontiguousarray(
                np.concatenate(
                    [np.asarray(in_maps[c][k]) for c in range(n)], axis=0
                )
            )
            for k in self.in_names
        ]
        self._resident = [
            self.jax.device_put(x, self.sharding) for x in concat_in
        ]
        self.jax.block_until_ready(self._resident)

    def _zeros(self):
        n = self.n_cores
        return [
            self.jax.device_put(
                np.zeros((n * z.shape[0], *z.shape[1:]), z.dtype), self.sharding
            )
            for z in self.zero_outs
        ]

    def run(self):
        outs = self.fn(*self._resident, *self._zeros())
        self.jax.block_until_ready(outs)
        n = self.n_cores
        return [
            {
                k: np.asarray(outs[i]).reshape(n, *self.out_avals[i].shape)[c]
                for i, k in enumerate(self.out_names)
            }
            for c in range(n)
        ]

    def time_s(self, reps=20, warmup=3):
        import time

        for _ in range(warmup):
            self.jax.block_until_ready(self.fn(*self._resident, *self._zeros()))
        times = []
        for _ in range(reps):
            z = self._zeros()
            t0 = time.perf_counter()
            out = self.fn(*self._resident, *z)
            self.jax.block_until_ready(out)
            times.append(time.perf_counter() - t0)
        return float(np.min(times))


_CACHE = {}


def get_compiled(inputs, reps=1):
    t_q = _pick_t_q(inputs)
    flags = _flags(inputs)
    key = (reps, t_q, flags)
    if key not in _CACHE:
        nc = build_nc(reps, t_q, *flags)
        _CACHE[key] = (CompiledSpmd(nc, NCORES), t_q)
    return _CACHE[key]


def kernel(**inputs) -> np.ndarray:
    comp, t_q = get_compiled(inputs, reps=1)
    in_maps = prep_in_maps(inputs, t_q)
    comp.stage_inputs(in_maps)
    res = comp.run()
    out = np.concatenate([res[c]["out"][0, :PSG_SH] for c in range(NCORES)])
    return out.astype(np.float32)


# revision 44
# speedup vs baseline: 1.9890x; 1.1202x over previous
"""Trainium2 Bass kernel for AuditableHybridGNN (gnn_message_passing).

Computation (reference):
  h_ent = LN((1-a)*h_local + a*h_global) * gamma_e + beta_e        [100000,256]
  rel   = sigmoid(sum(h_ent * q, -1))                              [100000]
  ctx   = segment_sum(h_ent[ent_idx] * rel[ent_idx], psg_idx)      [20000,256]
  h_psg = LN(h_passage + ctx) * gamma_p + beta_p                   [20000,256]
  out   = relu([h_psg, q] @ W1 + b1) @ W2 + b2                     [20000]

Distribution over 8 NeuronCores (SPMD, one program):
  - entities sharded 12500/core: each core computes g = h_ent*rel (bf16) for
    its shard, in 4 row-quarters; after each quarter an AllGather ships it to
    every core (4 chunked AGs overlap the entity phase);
  - edges sharded by PASSAGE owner: each core owns 2500 passages (20 blocks
    of 125) and consumes every edge pointing at them.  Edges are bucketed by
    (entity-quarter k, local block b); the gathered table for quarter k is
    the AG output slab (8*quarter rows, int16-indexable).  Per (k,b): one
    dma_gather of t_q*128 edge rows, then a one-hot [edge x 125] matmul
    accumulating into PSUM; k-partials are summed into an SBUF ctx buffer.
    No ReduceScatter and no DRAM round-trip for the segment sums.
  - per block: LN(h_passage + ctx) + scoring MLP -> 125 scores; host
    concatenates the per-core [2500] outputs.
"""
import contextlib
import os
import sys

sys.path.insert(0, "/opt/trn_rl_repo")

import ml_dtypes
import numpy as np

import concourse.bass as bass
from concourse import bacc, mybir, tile
from concourse.masks import make_identity

F32 = mybir.dt.float32
BF16 = mybir.dt.bfloat16
I16 = mybir.dt.int16

NCORES = 8
N_ENT = 100000
N_PSG = 20000
N_EDGE = 500000
D = 256
ALPHA = 0.1
EPS = 1e-5

ENT_SH = N_ENT // NCORES  # 12500
PSG_SH = N_PSG // NCORES  # 2500
ENT_TILES = (ENT_SH + 127) // 128  # 98
ENT_PAD = ENT_TILES * 128  # 12544
BLK = 125
NBLK_L = PSG_SH // BLK  # 20 local blocks
NQUART = 4
QT = [25, 25, 24, 24]  # entity tiles per quarter (sum = 98)
Q_ROWS = [t * 128 for t in QT]  # [3200, 3200, 3072, 3072]
Q_START = [0, 3200, 6400, 9472]
SLAB_ROWS = [NCORES * r for r in Q_ROWS]  # max 25600 < int16 range
NQ = int(os.environ.get("BASS_NQ", "4"))  # SWDGE queues for gathers
ECHUNK = 8  # entity tiles per DMA batch (also Sigmoid act-table batch size)
GMAX = 8  # max tiles per dma_gather (1024-idx SWDGE ring)


def build_nc(reps, t_q, use_gamma_e, use_beta_e, use_gamma_p, use_beta_p, use_b2,
             phases=(1, 1, 1, 1)):
    """phases = (ent, edge, ag, mlp) repeat counts (for differential timing)."""
    nc = bacc.Bacc(
        "TRN2",
        target_bir_lowering=False,
        debug=False,
        num_devices=NCORES,
        num_swdge_queues=NQ,
    )
    n_gi = NQUART * NBLK_L  # 80 gather groups
    n_slots = n_gi * t_q * 128

    hl_ext = nc.dram_tensor("hl", [ENT_PAD, D], F32, kind="ExternalInput")
    hg_ext = nc.dram_tensor("hg", [ENT_PAD, D], F32, kind="ExternalInput")
    hp_ext = nc.dram_tensor("hp", [PSG_SH, D], F32, kind="ExternalInput")
    q_ext = nc.dram_tensor("q", [1, D], F32, kind="ExternalInput")
    idx_ext = nc.dram_tensor("idx16", [128, n_slots // 16], I16, kind="ExternalInput")
    ploc_ext = nc.dram_tensor("ploc", [128, n_slots // 128], BF16, kind="ExternalInput")
    iota_ext = nc.dram_tensor("iota", [128, 128], BF16, kind="ExternalInput")
    w1_ext = nc.dram_tensor("w1", [D, D], F32, kind="ExternalInput")
    qb1_ext = nc.dram_tensor("qb1", [128, 2], F32, kind="ExternalInput")
    w2_ext = nc.dram_tensor("w2", [128, 2], F32, kind="ExternalInput")
    b2_ext = nc.dram_tensor("b2", [1, 1], F32, kind="ExternalInput")
    ge_ext = nc.dram_tensor("gamma_e", [1, D], F32, kind="ExternalInput")
    be_ext = nc.dram_tensor("beta_e", [1, D], F32, kind="ExternalInput")
    gp_ext = nc.dram_tensor("gamma_p", [1, D], F32, kind="ExternalInput")
    bp_ext = nc.dram_tensor("beta_p", [1, D], F32, kind="ExternalInput")
    out_ext = nc.dram_tensor("out", [1, PSG_SH], F32, kind="ExternalOutput")

    hl_r = hl_ext[:].rearrange("(c p) d -> p c d", p=128)
    hg_r = hg_ext[:].rearrange("(c p) d -> p c d", p=128)

    with tile.TileContext(nc) as tc:
        with (
            tc.tile_pool(name="consts", bufs=1) as consts,
            tc.tile_pool(name="ent", bufs=3) as ent,
            tc.tile_pool(name="ent_sc", bufs=4) as ent_sc,
            tc.tile_pool(name="gatp", bufs=20) as gatp,
            tc.tile_pool(name="ohp", bufs=6) as ohp,
            tc.tile_pool(name="ctxp", bufs=1) as ctxp,
            tc.tile_pool(name="mlp", bufs=3) as mlp,
            tc.tile_pool(name="eps_ps", bufs=3, space="PSUM") as eps_ps,
            tc.tile_pool(name="tp_ps", bufs=2, space="PSUM") as tp_ps,
            tc.tile_pool(name="hid_ps", bufs=2, space="PSUM") as hid_ps,
            tc.tile_pool(name="sc_ps", bufs=1, space="PSUM") as sc_ps,
            tc.tile_pool(name="dram", bufs=1, space="DRAM") as dram,
        ):
            # ---- constants loaded once ----
            idx_t = consts.tile([128, n_slots // 16], I16)
            nc.sync.dma_start(out=idx_t[:], in_=idx_ext[:])
            ploc_t = consts.tile([128, n_slots // 128], BF16)
            nc.sync.dma_start(out=ploc_t[:], in_=ploc_ext[:])
            iota_t = consts.tile([128, 128], BF16)
            nc.sync.dma_start(out=iota_t[:], in_=iota_ext[:])
            q_t = consts.tile([128, D], F32)
            nc.sync.dma_start(out=q_t[:], in_=q_ext[0:1, :].to_broadcast([128, D]))
            w1a_t = consts.tile([128, D], F32)
            nc.sync.dma_start(out=w1a_t[:], in_=w1_ext[0:128, :])
            w1b_t = consts.tile([128, D], F32)
            nc.sync.dma_start(out=w1b_t[:], in_=w1_ext[128:256, :])
            qb1_t = consts.tile([128, 2], F32)
            nc.sync.dma_start(out=qb1_t[:], in_=qb1_ext[:])
            w2_t = consts.tile([128, 2], F32)
            nc.sync.dma_start(out=w2_t[:], in_=w2_ext[:])
            b2_t = consts.tile([1, 1], F32)
            nc.sync.dma_start(out=b2_t[:], in_=b2_ext[:])
            eps_t = consts.tile([128, 1], F32)
            nc.vector.memset(eps_t[:], EPS)
            ident_t = consts.tile([128, 128], F32)
            make_identity(nc, ident_t[:])
            bc = {}
            for used, name, ext in (
                (use_gamma_e, "gamma_e", ge_ext),
                (use_beta_e, "beta_e", be_ext),
                (use_gamma_p, "gamma_p", gp_ext),
                (use_beta_p, "beta_p", bp_ext),
            ):
                if used:
                    t = consts.tile([128, D], F32, name=f"{name}_t")
                    nc.sync.dma_start(
                        out=t[:], in_=ext[0:1, :].to_broadcast([128, D])
                    )
                    bc[name] = t

            g_qs = [
                dram.tile([Q_ROWS[k], D], BF16, name=f"g_q{k}") for k in range(NQUART)
            ]
            g_q_rs = [
                g_qs[k][:].rearrange("(c p) d -> p c d", p=128) for k in range(NQUART)
            ]
            slabs = [
                dram.tile([SLAB_ROWS[k], D], BF16, name=f"slab{k}")
                for k in range(NQUART)
            ]

            use_scopes = bool(os.environ.get("BASS_SCOPES"))

            @contextlib.contextmanager
            def scope(name):
                if use_scopes:
                    with nc.named_scope(name):
                        yield
                else:
                    yield

            def s_enter(name):
                return nc.enter_named_scope(name, False)[0] if use_scopes else None

            def s_leave(name, sid):
                if sid is not None:
                    nc.leave_named_scope(name, sid, False)

            def layer_norm(x_ap, rows, gamma, beta):
                stats = ent_sc.tile([128, 6], F32, name="ln_stats")
                mv = ent_sc.tile([128, 2], F32, name="ln_mv")
                nc.vector.bn_stats(out=stats[:rows], in_=x_ap[:rows])
                nc.vector.bn_aggr(out=mv[:rows], in_=stats[:rows])
                nc.scalar.activation(
                    out=mv[:rows, 1:2],
                    in_=mv[:rows, 1:2],
                    func=mybir.ActivationFunctionType.Sqrt,
                    bias=eps_t[:rows],
                )
                nc.vector.reciprocal(out=mv[:rows, 1:2], in_=mv[:rows, 1:2])
                nc.vector.tensor_scalar(
                    out=x_ap[:rows],
                    in0=x_ap[:rows],
                    scalar1=mv[:rows, 0:1],
                    scalar2=mv[:rows, 1:2],
                    op0=mybir.AluOpType.subtract,
                    op1=mybir.AluOpType.mult,
                )
                if gamma is not None:
                    nc.vector.tensor_mul(x_ap[:rows], x_ap[:rows], gamma[:rows])
                if beta is not None:
                    nc.vector.tensor_add(x_ap[:rows], x_ap[:rows], beta[:rows])

            # tile index -> (quarter, tile offset within quarter)
            def quart_of(tile_idx):
                for k in range(NQUART):
                    if tile_idx < sum(QT[: k + 1]):
                        return k, tile_idx - sum(QT[:k])
                raise AssertionError

            def emit_ag(k):
                for _ in range(phases[2]):
                    with scope("ag"):
                        nc.gpsimd.collective_compute(
                            "AllGather",
                            mybir.AluOpType.bypass,
                            replica_groups=[list(range(NCORES))],
                            ins=[g_qs[k][:].opt()],
                            outs=[slabs[k][:].opt()],
                        )

            for _rep in range(reps):
                # ================= entity phase + chunked AllGathers ========
                gats = {}

                def emit_gathers(k):
                    for b in range(NBLK_L):
                        gi = k * NBLK_L + b
                        gat = gatp.tile([128, t_q, D], BF16, name="gat")
                        for t0 in range(0, t_q, GMAX):
                            nt = min(GMAX, t_q - t0)
                            c0 = (gi * t_q + t0) * 8  # idx16 col offset (16/col)
                            nc.gpsimd.dma_gather(
                                out_ap=gat[:, t0 : t0 + nt, :],
                                in_ap=slabs[k][:],
                                idxs_ap=idx_t[:, c0 : c0 + nt * 8],
                                num_idxs=nt * 128,
                                num_idxs_reg=nt * 128,
                                elem_size=D,
                                queue_num=gi % NQ,
                            )
                        gats[(k, b)] = gat

                _se = s_enter("ent")
                ag_done = 0
                ent_iters = [
                    c for _ in range(phases[0]) for c in range(0, ENT_TILES, ECHUNK)
                ]
                for c0 in ent_iters:
                    ntc = min(ECHUNK, ENT_TILES - c0)
                    hl_t = ent.tile([128, ECHUNK, D], F32, name="hl_t")
                    hg_t = ent.tile([128, ECHUNK, D], F32, name="hg_t")
                    g_t = ent.tile([128, ECHUNK, D], BF16, name="g_t")
                    nc.sync.dma_start(
                        out=hl_t[:, :ntc, :], in_=hl_r[:, c0 : c0 + ntc, :]
                    )
                    nc.sync.dma_start(
                        out=hg_t[:, :ntc, :], in_=hg_r[:, c0 : c0 + ntc, :]
                    )
                    relc = ent_sc.tile([128, ECHUNK], F32, name="relc")
                    for j in range(ntc):
                        a = hl_t[:, j, :]
                        b = hg_t[:, j, :]
                        # LN is scale-invariant: LN(.9*hl+.1*hg) == LN(hl+(1/9)*hg)
                        nc.scalar.activation(
                            out=b, in_=b,
                            func=mybir.ActivationFunctionType.Copy,
                            scale=ALPHA / (1.0 - ALPHA),
                        )
                        nc.vector.tensor_add(a, a, b)
                        layer_norm(a, 128, bc.get("gamma_e"), bc.get("beta_e"))
                        xq = ent_sc.tile([128, D], F32, name="xq")
                        nc.vector.tensor_mul(xq[:], a, q_t[:])
                        nc.vector.reduce_sum(
                            out=relc[:, j : j + 1],
                            in_=xq[:],
                            axis=mybir.AxisListType.X,
                        )
                    nc.scalar.activation(
                        out=relc[:, :ntc],
                        in_=relc[:, :ntc],
                        func=mybir.ActivationFunctionType.Sigmoid,
                    )
                    for j in range(ntc):
                        nc.scalar.activation(
                            out=g_t[:, j, :],
                            in_=hl_t[:, j, :],
                            func=mybir.ActivationFunctionType.Copy,
                            scale=relc[:, j : j + 1],
                        )
                    # write g tiles to their quarter tensors (may straddle)
                    j0 = 0
                    while j0 < ntc:
                        k, off = quart_of((c0 + j0) % ENT_TILES)
                        nsp = min(ntc - j0, QT[k] - off)
                        nc.sync.dma_start(
                            out=g_q_rs[k][:, off : off + nsp, :],
                            in_=g_t[:, j0 : j0 + nsp, :],
                        )
                        j0 += nsp
                    # fire AG for completed quarters
                    last_tile = (c0 % ENT_TILES) + ntc
                    while ag_done < NQUART and last_tile >= sum(QT[: ag_done + 1]):
                        emit_ag(ag_done)
                        if phases[1]:
                            emit_gathers(ag_done)
                        ag_done += 1
                if not phases[0]:
                    for k in range(NQUART):
                        emit_ag(k)
                        if phases[1]:
                            emit_gathers(k)
                s_leave("ent", _se)

                # ============ edge phase: gather + one-hot matmul ===========
                ctx_sb = ctxp.tile([128, NBLK_L, D], F32, name="ctx_sb")
                if not phases[1]:  # timing configs only: keep ctx_sb readable
                    nc.vector.memset(ctx_sb[:], 0.0)
                score_sb = mlp.tile([1, PSG_SH], F32, name="score_sb")

                # ================= per-block LN + scoring MLP ===============
                def mlp_blk(b):
                    _sm = s_enter("mlp")
                    hx = mlp.tile([128, D], F32, name="hx")
                    nc.scalar.dma_start(
                        out=hx[:BLK, :], in_=hp_ext[b * BLK : (b + 1) * BLK, :]
                    )
                    nc.vector.tensor_add(hx[:BLK, :], hx[:BLK, :], ctx_sb[:BLK, b, :])
                    layer_norm(hx, BLK, bc.get("gamma_p"), bc.get("beta_p"))
                    xts = []
                    for dc in range(2):
                        tp = tp_ps.tile([128, 128], F32, space="PSUM", name="tp")
                        nc.tensor.transpose(
                            out=tp[:, :BLK],
                            in_=hx[:BLK, dc * 128 : (dc + 1) * 128],
                            identity=ident_t[:BLK, :BLK],
                        )
                        xt = mlp.tile([128, 128], F32, name="xt")
                        nc.scalar.copy(out=xt[:, :BLK], in_=tp[:, :BLK])
                        xts.append(xt)
                    hids = []
                    for jc in range(2):
                        hp_ = hid_ps.tile([128, 128], F32, space="PSUM", name="hp_")
                        nc.tensor.matmul(
                            out=hp_[:, :BLK],
                            lhsT=w1a_t[:, jc * 128 : (jc + 1) * 128],
                            rhs=xts[0][:, :BLK],
                            start=True,
                            stop=False,
                        )
                        nc.tensor.matmul(
                            out=hp_[:, :BLK],
                            lhsT=w1b_t[:, jc * 128 : (jc + 1) * 128],
                            rhs=xts[1][:, :BLK],
                            start=False,
                            stop=True,
                        )
                        hid = mlp.tile([128, 128], F32, name="hid")
                        nc.scalar.activation(
                            out=hid[:, :BLK],
                            in_=hp_[:, :BLK],
                            func=mybir.ActivationFunctionType.Relu,
                            bias=qb1_t[:, jc : jc + 1],
                        )
                        hids.append(hid)
                    sc = sc_ps.tile([1, 128], F32, space="PSUM", name="sc")
                    nc.tensor.matmul(
                        out=sc[:, :BLK], lhsT=w2_t[:, 0:1], rhs=hids[0][:, :BLK],
                        start=True, stop=False,
                    )
                    nc.tensor.matmul(
                        out=sc[:, :BLK], lhsT=w2_t[:, 1:2], rhs=hids[1][:, :BLK],
                        start=False, stop=True,
                    )
                    if use_b2:
                        nc.vector.tensor_scalar(
                            out=score_sb[:, b * BLK : (b + 1) * BLK],
                            in0=sc[:, :BLK],
                            scalar1=b2_t[0:1, 0:1],
                            scalar2=None,
                            op0=mybir.AluOpType.add,
                        )
                    else:
                        nc.vector.tensor_copy(
                            out=score_sb[:, b * BLK : (b + 1) * BLK], in_=sc[:, :BLK]
                        )
                    s_leave("mlp", _sm)


                edge_iters = [
                    (k, b)
                    for _ in range(phases[1])
                    for k in range(NQUART)
                    for b in range(NBLK_L)
                ]
                _sg = s_enter("edge")
                for k, b in edge_iters:
                    gi = k * NBLK_L + b
                    gat = gats[(k, b)]
                    acc = eps_ps.tile([128, D], F32, space="PSUM", name="acc")
                    for t in range(t_q):
                        col = gi * t_q + t
                        oh = ohp.tile([128, BLK], BF16, name="oh")
                        nc.vector.tensor_tensor(
                            out=oh[:],
                            in0=ploc_t[:, col : col + 1].to_broadcast([128, BLK]),
                            in1=iota_t[:, :BLK],
                            op=mybir.AluOpType.is_equal,
                        )
                        nc.tensor.matmul(
                            out=acc[:BLK, :],
                            lhsT=oh[:],
                            rhs=gat[:, t, :],
                            start=(t == 0),
                            stop=(t == t_q - 1),
                        )
                    if k == 0:
                        nc.scalar.copy(out=ctx_sb[:BLK, b, :], in_=acc[:BLK, :])
                    else:
                        nc.vector.tensor_add(
                            ctx_sb[:BLK, b, :], ctx_sb[:BLK, b, :], acc[:BLK, :]
                        )
                    if k == NQUART - 1 and phases[3]:
                        for _ in range(phases[3]):
                            mlp_blk(b)
                s_leave("edge", _sg)

                if not phases[3]:  # timing configs only
                    nc.vector.memset(score_sb[:], 0.0)
                if phases[3] and not phases[1]:
                    for _ in range(phases[3]):
                        for b in range(NBLK_L):
                            mlp_blk(b)
                nc.sync.dma_start(out=out_ext[:], in_=score_sb[:])
    nc.compile()
    return nc


# ---------------------------------------------------------------------------
# host-side input prep
# ---------------------------------------------------------------------------
def pack_idx16(idx: np.ndarray) -> np.ndarray:
    n = idx.shape[0]
    arr = idx.astype(np.int16).reshape(n // 16, 16).T  # [16, n/16]
    return np.ascontiguousarray(np.tile(arr, (8, 1)))  # [128, n/16]


def _edge_keys(ent_idx, psg_idx, r):
    """Return (sel, gi, slabrow, pin) for core r's edges."""
    lo = r * PSG_SH
    sel = np.nonzero((psg_idx >= lo) & (psg_idx < lo + PSG_SH))[0]
    p = (psg_idx[sel] - lo).astype(np.int64)
    e = ent_idx[sel].astype(np.int64)
    b = p // BLK
    pin = p % BLK
    owner = e // ENT_SH
    l = e % ENT_SH
    qk = np.searchsorted(np.asarray(Q_START[1:]), l, side="right")
    qrows = np.asarray(Q_ROWS)[qk]
    qstart = np.asarray(Q_START)[qk]
    slabrow = owner * qrows + (l - qstart)
    gi = qk * NBLK_L + b
    return sel, gi, slabrow, pin


def _pick_t_q(inputs):
    ent_idx = np.asarray(inputs["ent_idx"])
    psg_idx = np.asarray(inputs["psg_idx"])
    mx = 0
    for r in range(NCORES):
        _, gi, _, _ = _edge_keys(ent_idx, psg_idx, r)
        cnt = np.bincount(gi, minlength=NQUART * NBLK_L)
        mx = max(mx, int(cnt.max()))
    return (mx + 127) // 128


def prep_in_maps(inputs: dict, t_q: int) -> list[dict]:
    ent_idx = np.asarray(inputs["ent_idx"])
    psg_idx = np.asarray(inputs["psg_idx"])
    hl = np.asarray(inputs["h_local_ent"])
    hg = np.asarray(inputs["h_ent_global"])
    hp = np.asarray(inputs["h_passage"])
    q = np.asarray(inputs["query_emb"]).reshape(1, D)
    w1 = np.asarray(inputs["W1"])
    b1 = np.asarray(inputs["b1"])
    w2 = np.asarray(inputs["W2"])
    b2 = np.asarray(inputs["b2"])

    qb1 = (q[0] @ w1[D:]) + b1  # [256]
    iota = np.ascontiguousarray(
        np.tile(np.arange(128, dtype=np.float32)[None, :], (128, 1))
    ).astype(ml_dtypes.bfloat16)
    n_gi = NQUART * NBLK_L
    n_slots = n_gi * t_q * 128
    tpg = t_q * 128

    in_maps = []
    for r in range(NCORES):
        _, gi, slabrow, pin = _edge_keys(ent_idx, psg_idx, r)
        order = np.argsort(gi, kind="stable")
        gi = gi[order]
        slabrow = slabrow[order]
        pin = pin[order]
        counts = np.bincount(gi, minlength=n_gi)
        assert counts.max() <= tpg, (counts.max(), tpg)
        starts = np.zeros(n_gi, np.int64)
        np.cumsum(counts[:-1], out=starts[1:])
        rank = np.arange(len(gi)) - starts[gi]
        pos = gi * tpg + rank
        slots_e = np.zeros(n_slots, np.int16)
        slots_p = np.full(n_slots, 300.0, np.float32)
        slots_e[pos] = slabrow.astype(np.int16)
        slots_p[pos] = pin.astype(np.float32)

        lo = r * ENT_SH
        hl_sh = np.zeros((ENT_PAD, D), np.float32)
        hl_sh[:ENT_SH] = hl[lo : lo + ENT_SH]
        hg_sh = np.zeros((ENT_PAD, D), np.float32)
        hg_sh[:ENT_SH] = hg[lo : lo + ENT_SH]
        hp_sh = np.ascontiguousarray(hp[r * PSG_SH : (r + 1) * PSG_SH])

        in_maps.append(
            {
                "hl": hl_sh,
                "hg": hg_sh,
                "hp": hp_sh,
                "q": q.astype(np.float32),
                "idx16": pack_idx16(slots_e),
                "ploc": np.ascontiguousarray(
                    slots_p.reshape(n_slots // 128, 128).T
                ).astype(ml_dtypes.bfloat16),
                "iota": iota,
                "w1": np.ascontiguousarray(w1[:D]),
                "qb1": np.ascontiguousarray(qb1.reshape(2, 128).T),
                "w2": np.ascontiguousarray(w2[:, 0].reshape(2, 128).T),
                "b2": b2.reshape(1, 1).astype(np.float32),
                "gamma_e": np.asarray(inputs["gamma_e"]).reshape(1, D),
                "beta_e": np.asarray(inputs["beta_e"]).reshape(1, D),
                "gamma_p": np.asarray(inputs["gamma_p"]).reshape(1, D),
                "beta_p": np.asarray(inputs["beta_p"]).reshape(1, D),
            }
        )
    return in_maps


def _flags(inputs):
    return (
        not np.all(np.asarray(inputs["gamma_e"]) == 1.0),
        not np.all(np.asarray(inputs["beta_e"]) == 0.0),
        not np.all(np.asarray(inputs["gamma_p"]) == 1.0),
        not np.all(np.asarray(inputs["beta_p"]) == 0.0),
        not np.all(np.asarray(inputs["b2"]) == 0.0),
    )


# backwards-compat alias used by test.py
_pick_t_b = _pick_t_q


# ---------------------------------------------------------------------------
# PJRT SPMD execution (axon)
# ---------------------------------------------------------------------------
class CompiledSpmd:
    def __init__(self, nc, n_cores: int):
        import jax
        from jax.experimental.shard_map import shard_map
        from jax.sharding import Mesh, NamedSharding, PartitionSpec

        from concourse.bass2jax import (
            _bass_exec_p,
            install_neuronx_cc_hook,
            partition_id_tensor,
        )

        self.jax = jax
        install_neuronx_cc_hook()
        self.n_cores = n_cores
        partition_name = (
            nc.partition_id_tensor.name if nc.partition_id_tensor else None
        )
        in_names, out_names, out_avals, zero_outs = [], [], [], []
        for alloc in nc.m.functions[0].allocations:
            if not isinstance(alloc, mybir.MemoryLocationSet):
                continue
            name = alloc.memorylocations[0].name
            if alloc.kind == "ExternalInput":
                if name != partition_name:
                    in_names.append(name)
            elif alloc.kind == "ExternalOutput":
                out_names.append(name)
                shape = tuple(alloc.tensor_shape)
                dtype = mybir.dt.np(alloc.dtype)
                out_avals.append(jax.core.ShapedArray(shape, dtype))
                zero_outs.append(np.zeros(shape, dtype))
        self.in_names = in_names
        self.out_names = out_names
        self.out_avals = out_avals
        self.zero_outs = zero_outs
        n_params = len(in_names)
        n_outs = len(out_avals)
        all_in_names = in_names + out_names
        if partition_name is not None:
            all_in_names.append(partition_name)
        donate = tuple(range(n_params, n_params + n_outs))

        def _body(*args):
            operands = list(args)
            if partition_name is not None:
                operands.append(partition_id_tensor())
            outs = _bass_exec_p.bind(
                *operands,
                out_avals=tuple(out_avals),
                in_names=tuple(all_in_names),
                out_names=tuple(out_names),
                lowering_input_output_aliases=(),
                sim_require_finite=True,
                sim_require_nnan=True,
                nc=nc,
            )
            return tuple(outs)

        devices = jax.devices()[:n_cores]
        assert len(devices) == n_cores
        self.mesh = Mesh(np.asarray(devices), ("core",))
        in_specs = (PartitionSpec("core"),) * (n_params + n_outs)
        out_specs = (PartitionSpec("core"),) * len(out_names)
        self.sharding = NamedSharding(self.mesh, PartitionSpec("core"))
        self.fn = jax.jit(
            shard_map(
                _body,
                mesh=self.mesh,
                in_specs=in_specs,
                out_specs=out_specs,
                check_rep=False,
            ),
            donate_argnums=donate,
            keep_unused=True,
        )
        self._resident = None

    def stage_inputs(self, in_maps):
        n = self.n_cores
        concat_in = [
            np.ascontiguousarray(
                np.concatenate(
                    [np.asarray(in_maps[c][k]) for c in range(n)], axis=0
                )
            )
            for k in self.in_names
        ]
        self._resident = [
            self.jax.device_put(x, self.sharding) for x in concat_in
        ]
        self.jax.block_until_ready(self._resident)

    def _zeros(self):
        n = self.n_cores
        return [
            self.jax.device_put(
                np.zeros((n * z.shape[0], *z.shape[1:]), z.dtype), self.sharding
            )
            for z in self.zero_outs
        ]

    def run(self):
        outs = self.fn(*self._resident, *self._zeros())
        self.jax.block_until_ready(outs)
        n = self.n_cores
        return [
            {
                k: np.asarray(outs[i]).reshape(n, *self.out_avals[i].shape)[c]
                for i, k in enumerate(self.out_names)
            }
            for c in range(n)
        ]

    def time_s(self, reps=20, warmup=3):
        import time

        for _ in range(warmup):
            self.jax.block_until_ready(self.fn(*self._resident, *self._zeros()))
        times = []
        for _ in range(reps):
            z = self._zeros()
            t0 = time.perf_counter()
            out = self.fn(*self._resident, *z)
            self.jax.block_until_ready(out)
            times.append(time.perf_counter() - t0)
        return float(np.min(times))


_CACHE = {}


def get_compiled(inputs, reps=1):
    t_q = _pick_t_q(inputs)
    flags = _flags(inputs)
    key = (reps, t_q, flags)
    if key not in _CACHE:
        nc = build_nc(reps, t_q, *flags)
        _CACHE[key] = (CompiledSpmd(nc, NCORES), t_q)
    return _CACHE[key]


def kernel(**inputs) -> np.ndarray:
    comp, t_q = get_compiled(inputs, reps=1)
    in_maps = prep_in_maps(inputs, t_q)
    comp.stage_inputs(in_maps)
    res = comp.run()
    out = np.concatenate([res[c]["out"][0, :PSG_SH] for c in range(NCORES)])
    return out.astype(np.float32)
